# revision 66
# baseline (speedup 1.0000x reference)
"""Trainium2 8-core kernel for the ConvFF + BatchNorm + Mamba block.

Sharding (8 NeuronCores): core i -> b = i//4 (batch), q = i%4.
  - Front (ff conv + BN): computes output-channel tile q (128 of 512
    channels) for batch b.  The RAW conv output is AllGathered within
    each b-group; BN stats go through one 8-core AllReduce (each core
    contributes its tile's stats into a one-hot column slot) and
    normalization happens on the consumer side, off the CC path.
  - Mamba: d_inner slice q (256 of 1024 channels) for batch b; the
    x_proj partial is all-reduced within the b-group; the selective
    scan runs fully local via the DVE tensor_tensor_scan instruction
    (h_t = dA_t * h_{t-1} + dBx_t along the free/time axis).
  - out_proj partials reduce-scattered within the b-group back to
    channel tile q; each core emits its [128, 2048] output shard.

The sequence is processed as a 3-part pipeline [512, 1024, 512]: a
small first part so the scan starts as early as possible behind the
collective chain, and a small last part so the final ReduceScatter
exposes as little latency as possible.  A dummy AllReduce issued at
t=0 absorbs the ~60-70us CC-subsystem init.

Engine roles in the scan phase: DVE runs everything elementwise
(scan + bf16 2x multiplies), Act runs the exps, PE accumulates
y = sum_n C_n*h_n via identity matmuls (opened with a diag(D_skip)
matmul so the skip term is free), and the B/C row broadcasts are
stride-0 DMA reads from the AllReduce DRAM output.  Pool runs ONLY
light/off-phase work: its SBUF-read-heavy ops would arbitrate away
the DVE scan's second read port (2x scan slowdown).
"""

import os
import sys
import numpy as np

for _p in ("/opt/trn_rl_repo", "/root/.axon_site/_ro/trn_rl_repo"):
    if os.path.isdir(_p) and _p not in sys.path:
        sys.path.append(_p)

import ml_dtypes  # noqa: E402

from concourse import bass, bacc, mybir, tile  # noqa: E402
from concourse.bass_utils import run_bass_kernel_spmd  # noqa: E402

F32 = mybir.dt.float32
BF16 = mybir.dt.bfloat16
AF = mybir.ActivationFunctionType
OP = mybir.AluOpType

B, L, C, DI, N, RK, KK, DC = 2, 2048, 512, 1024, 16, 32, 7, 4
S = DI // 4      # 256 d_inner channels per core
CT = C // 4      # 128 output channels per core
P = 128
LB = 512         # l-block (one PSUM bank of f32)
NLB = L // LB
EPS = 1e-5

PARTS = [(0, 512), (512, 1024), (1536, 512)]   # (offset, size) pipeline
NPART = len(PARTS)

GRP_B = [[0, 1, 2, 3], [4, 5, 6, 7]]        # b-groups
GRP_PAIR = [[0, 4], [1, 5], [2, 6], [3, 7]]
GRP_ALL = [[0, 1, 2, 3, 4, 5, 6, 7]]


def build_graph():
    nc = bacc.Bacc("TRN2", target_bir_lowering=False, debug=False,
                   num_devices=8)

    # ---- kernel I/O --------------------------------------------------
    xb = nc.dram_tensor("xb", [C, L], BF16, kind="ExternalInput")
    xct = nc.dram_tensor("xct", [CT, L], F32, kind="ExternalInput")
    ffw = nc.dram_tensor("ffw", [KK, C, CT], BF16, kind="ExternalInput")
    ffb = nc.dram_tensor("ffb", [CT, 1], F32, kind="ExternalInput")
    gamma = nc.dram_tensor("gamma", [CT, 4], F32, kind="ExternalInput")
    beta = nc.dram_tensor("beta", [CT, 4], F32, kind="ExternalInput")
    qsel = nc.dram_tensor("qsel", [CT, 4], F32, kind="ExternalInput")
    winx = nc.dram_tensor("winx", [C, S], BF16, kind="ExternalInput")
    winz = nc.dram_tensor("winz", [C, S], BF16, kind="ExternalInput")
    convd = nc.dram_tensor("convd", [DC, S, P], BF16, kind="ExternalInput")
    convb = nc.dram_tensor("convb", [S, 1], F32, kind="ExternalInput")
    wxp = nc.dram_tensor("wxp", [S, RK + 2 * N], BF16, kind="ExternalInput")
    wdt = nc.dram_tensor("wdt", [RK, S], BF16, kind="ExternalInput")
    dtb = nc.dram_tensor("dtb", [S, 1], F32, kind="ExternalInput")
    acol = nc.dram_tensor("acol", [S, N], F32, kind="ExternalInput")
    dskipd = nc.dram_tensor("dskipd", [S, P], BF16, kind="ExternalInput")
    wout = nc.dram_tensor("wout", [S, C], BF16, kind="ExternalInput")
    ident = nc.dram_tensor("ident", [P, P], BF16, kind="ExternalInput")
    out = nc.dram_tensor("out", [CT, L], F32, kind="ExternalOutput")

    with tile.TileContext(nc) as tc:
        _emit(nc, tc, xb, xct, ffw, ffb, gamma, beta, qsel, winx, winz,
              convd, convb, wxp, wdt, dtb, acol, dskipd, wout, ident, out)

    nc.compile()
    return nc


def _emit(nc, tc, xb, xct, ffw, ffb, gamma, beta, qsel, winx, winz, convd,
          convb, wxp, wdt, dtb, acol, dskipd, wout, ident, out):
    sync = nc.sync
    vec = nc.vector
    act = nc.scalar
    pe = nc.tensor
    gps = nc.gpsimd

    import contextlib
    _ctx = contextlib.ExitStack()
    _pers = _ctx.enter_context(tc.tile_pool(name="pers", bufs=1))

    def stile(shape, dtype, name):
        return _pers.tile(shape, dtype, name=name, tag=name)

    # ---- DRAM bounce buffers for collectives -------------------------
    dram = _ctx.enter_context(tc.tile_pool(name="dram", bufs=1, space="DRAM"))
    bn_in = dram.tile([CT, 8], F32, name="bn_in")
    bn_out = dram.tile([CT, 8], F32, name="bn_out")
    dum_in = dram.tile([1, 16], F32, name="dum_in")
    dum_out = dram.tile([1, 16], F32, name="dum_out")
    ng_in_p = [dram.tile([CT, sz], BF16, name=f"ng_in{p}")
               for p, (o, sz) in enumerate(PARTS)]
    ng_out_p = [dram.tile([C, sz], BF16, name=f"ng_out{p}")
                for p, (o, sz) in enumerate(PARTS)]
    dbc_in_p = [dram.tile([RK + 2 * N, sz], BF16, name=f"dbc_in{p}")
                for p, (o, sz) in enumerate(PARTS)]
    dbc_out_p = [dram.tile([RK + 2 * N, sz], BF16, name=f"dbc_out{p}")
                 for p, (o, sz) in enumerate(PARTS)]
    att_in_p = [dram.tile([C, sz], BF16, name=f"att_in{p}")
                for p, (o, sz) in enumerate(PARTS)]
    att_out_p = [dram.tile([CT, sz], BF16, name=f"att_out{p}")
                 for p, (o, sz) in enumerate(PARTS)]

    # ---- persistent SBUF tiles --------------------------------------
    # 4 KiB-multiple tiles are allocated first so every hot [P, L] tile
    # is bank-aligned: a tile sharing a 4 KiB SBUF bank with another
    # operand of the same (or a concurrent) instruction costs ~20% DVE
    # throughput.
    xct_sb = stile([CT, L], F32, "xct_sb")   # becomes res (= ff+x) in-place
    ff_out = stile([CT, L], F32, "ff_out")
    xs_act = [stile([P, L], BF16, f"xs_act{d}") for d in range(2)]
    zsil = [stile([P, L], BF16, f"zsil{d}") for d in range(2)]
    dtT = [stile([P, L], BF16, f"dtT{d}") for d in range(2)]
    vT = [stile([P, L], BF16, f"vT{d}") for d in range(2)]
    yg = [stile([P, L], BF16, f"yg{d}") for d in range(2)]

    ffb_sb = stile([CT, 1], F32, "ffb_sb")
    gamma_sb = stile([CT, 4], F32, "gamma_sb")
    beta_sb = stile([CT, 4], F32, "beta_sb")
    qsel_sb = stile([CT, 4], F32, "qsel_sb")
    wxp_sb = stile([P, 2 * (RK + 2 * N)], BF16, "wxp_sb")
    wdt_sb = stile([RK, S], BF16, "wdt_sb")
    dtb_sb = stile([P, 2], F32, "dtb_sb")
    acol_sb = stile([P, 2 * N], F32, "acol_sb")
    dskipd_sb = stile([P, 2 * P], BF16, "dskipd_sb")
    convd_sb = stile([P, DC * 2 * P], BF16, "convd_sb")
    convb_sb = stile([P, 2], F32, "convb_sb")
    wout_sb = stile([P, 8 * P], BF16, "wout_sb")
    idt_sb = stile([P, P], BF16, "idt_sb")
    hfin = stile([P, 2 * N], F32, "hfin")
    wsink = stile([P, 1], F32, "wsink")

    sync.dma_start(out=ffb_sb[:], in_=ffb.ap()[:, :])
    # dummy collective triggered at t~0: the CC subsystem takes ~55us
    # from its FIRST trigger to execute anything, so an early throwaway
    # trigger absorbs that init while the conv runs.  Pair groups keep
    # the dummy itself as short as possible (~6us vs ~11 for 8-core).
    gps.collective_compute("AllReduce", OP.add, replica_groups=GRP_ALL,
                           ins=[dum_in.opt()], outs=[dum_out.opt()])

    # =============================================================
    # Phase 1: ff conv (Conv1d k=7 same-pad) + ReLU -> ff_out,
    # BN partial stats accumulated during the PSUM evacuation, and a
    # raw bf16 copy staged out for the per-part AllGathers.
    # =============================================================
    ssum = stile([CT, 4], F32, "ssum")
    ssq = stile([CT, 4], F32, "ssq")
    with tc.tile_pool(name="ffpool", bufs=1) as ffp, \
         tc.tile_pool(name="sqpool", bufs=2) as sqp, \
         tc.tile_pool(name="ffpsum", bufs=3, space="PSUM") as ffps:
        # PE p-state warm-up: dummy matmuls during the input DMA wait so
        # the real conv starts at full clock (2.4 GHz).
        warm = ffp.tile([P, LB], BF16, name="warm")
        gps.memset(warm[:], 0.0)
        wps = ffps.tile([P, LB], F32, name="wps")
        for w in range(28):
            pe.matmul(out=wps[:], lhsT=warm[:, 0:P], rhs=warm[:],
                      start=(w == 0), stop=(w == 27),
                      skip_group_check=True)
        act.copy(wsink[:], wps[:, 0:1])
        ffraw16 = ffp.tile([CT, L], BF16, name="ffraw16")
        ffw_sb = ffp.tile([P, KK * 4 * P], BF16, name="ffw_sb")
        for ci in range(4):
            eng = (act, gps, sync, act)[ci]
            eng.dma_start(
                out=ffw_sb[:, ci * KK * P:(ci + 1) * KK * P]
                .rearrange("p (k m) -> p k m", k=KK),
                in_=ffw.ap()[:, ci * P:(ci + 1) * P, :]
                .rearrange("k p m -> p k m"))
        x_sb = []
        for ci in range(4):
            t = ffp.tile([P, L + 6], BF16, name=f"x_sb{ci}")
            gps.memset(t[:, 0:3], 0.0)
            gps.memset(t[:, L + 3:L + 6], 0.0)
            eng = (sync, act, gps, sync)[ci]
            eng.dma_start(out=t[:, 3:3 + L // 2],
                          in_=xb.ap()[ci * P:(ci + 1) * P, 0:L // 2])
            eng.dma_start(out=t[:, 3 + L // 2:3 + L],
                          in_=xb.ap()[ci * P:(ci + 1) * P, L // 2:L])
            x_sb.append(t)

        for lb in range(NLB):
            ps = ffps.tile([P, LB], F32, name="ffps")
            nmm = KK * 4
            j = 0
            for k in range(KK):
                for ci in range(4):
                    jj = ci * KK + k
                    pe.matmul(
                        out=ps[:],
                        lhsT=ffw_sb[:, jj * P:(jj + 1) * P],
                        rhs=x_sb[ci][:, k + lb * LB:k + lb * LB + LB],
                        start=(j == 0), stop=(j == nmm - 1))
                    j += 1
            act.activation(out=ff_out[:, lb * LB:(lb + 1) * LB], in_=ps[:],
                           func=AF.Relu, bias=ffb_sb[:, 0:1],
                           accum_out=ssum[:, lb:lb + 1])
            sq = sqp.tile([CT, LB], BF16, name="sq", bufs=2)
            act.activation(out=sq[:], in_=ff_out[:, lb * LB:(lb + 1) * LB],
                           func=AF.Square, accum_out=ssq[:, lb:lb + 1])
            act.copy(ffraw16[:, lb * LB:(lb + 1) * LB],
                     ff_out[:, lb * LB:(lb + 1) * LB])
            # stage each pipeline part's raw conv out as soon as ready
            for pp, (o, sz) in enumerate(PARTS):
                if o + sz == (lb + 1) * LB:
                    act.dma_start(out=ng_in_p[pp][:],
                                  in_=ffraw16[:, o:o + sz])

    # weight loads for the mamba part (DMA-idle window during conv)
    sync.dma_start(out=gamma_sb[:], in_=gamma.ap()[:, :])
    sync.dma_start(out=beta_sb[:], in_=beta.ap()[:, :])
    sync.dma_start(out=qsel_sb[:], in_=qsel.ap()[:, :])
    sync.dma_start(out=wdt_sb[:], in_=wdt.ap()[:, :])
    sync.dma_start(out=xct_sb[:], in_=xct.ap()[:, :])
    sync.dma_start(out=idt_sb[:], in_=ident.ap()[:, :])
    for d in range(2):
        rs = slice(d * P, (d + 1) * P)
        sync.dma_start(out=wxp_sb[:, d * 64:(d + 1) * 64],
                       in_=wxp.ap()[rs, :])
        sync.dma_start(out=dtb_sb[:, d:d + 1], in_=dtb.ap()[rs, :])
        sync.dma_start(out=acol_sb[:, d * N:(d + 1) * N],
                       in_=acol.ap()[rs, :])
        sync.dma_start(out=dskipd_sb[:, d * P:(d + 1) * P],
                       in_=dskipd.ap()[rs, :])
        sync.dma_start(out=convb_sb[:, d:d + 1], in_=convb.ap()[rs, :])
        sync.dma_start(out=wout_sb[:, d * 4 * P:(d + 1) * 4 * P],
                       in_=wout.ap()[rs, :])
        sync.dma_start(
            out=convd_sb[:, d * DC * P:(d + 1) * DC * P]
            .rearrange("p (k m) -> p k m", k=DC),
            in_=convd.ap()[:, d * P:(d + 1) * P, :]
            .rearrange("k p m -> p k m"))

    # =============================================================
    # Phase 2: BN stats via one 8-core AllReduce of one-hot-placed
    # per-tile stats; bn scale/shift chain for all 4 channel tiles.
    # =============================================================
    bscale4 = stile([CT, 4], F32, "bscale4")
    bshift4 = stile([CT, 4], F32, "bshift4")
    with tc.tile_pool(name="bnpool", bufs=1) as bnp:
        stot = bnp.tile([CT, 2], F32, name="stot")
        stat8 = bnp.tile([CT, 8], F32, name="stat8")
        stat2 = bnp.tile([CT, 8], F32, name="stat2")
        vec.tensor_reduce(out=stot[:, 0:1], in_=ssum[:],
                          axis=mybir.AxisListType.X, op=OP.add)
        vec.tensor_reduce(out=stot[:, 1:2], in_=ssq[:],
                          axis=mybir.AxisListType.X, op=OP.add)
        vec.tensor_tensor(out=stat8[:, 0:4], in0=qsel_sb[:],
                          in1=stot[:, 0:1].to_broadcast((CT, 4)),
                          op=OP.mult)
        vec.tensor_tensor(out=stat8[:, 4:8], in0=qsel_sb[:],
                          in1=stot[:, 1:2].to_broadcast((CT, 4)),
                          op=OP.mult)
        sync.dma_start(out=bn_in[:], in_=stat8[:])
        gps.collective_compute("AllReduce", OP.add,
                               replica_groups=GRP_ALL,
                               ins=[bn_in.opt()], outs=[bn_out.opt()])
        # residual precompute (in-place into xct_sb) on Pool while it
        # idles here — Pool must never run read-heavy ops during the
        # scan phase (its SBUF reads arbitrate away the DVE scan's
        # second read port, 2x slowdown)
        for h in range(2):
            gps.tensor_tensor(out=xct_sb[:, h * 1024:(h + 1) * 1024],
                              in0=ff_out[:, h * 1024:(h + 1) * 1024],
                              in1=xct_sb[:, h * 1024:(h + 1) * 1024],
                              op=OP.add)
        sync.dma_start(out=stat2[:], in_=bn_out[:])

        mu4 = bnp.tile([CT, 4], F32, name="mu4")
        ex24 = bnp.tile([CT, 4], F32, name="ex24")
        msq4 = bnp.tile([CT, 4], F32, name="msq4")
        var4 = bnp.tile([CT, 4], F32, name="var4")
        std4 = bnp.tile([CT, 4], F32, name="std4")
        rstd4 = bnp.tile([CT, 4], F32, name="rstd4")
        tmp4 = bnp.tile([CT, 4], F32, name="tmp4")
        vec.tensor_scalar_mul(out=mu4[:], in0=stat2[:, 0:4],
                              scalar1=1.0 / (B * L))
        vec.tensor_scalar_mul(out=ex24[:], in0=stat2[:, 4:8],
                              scalar1=1.0 / (B * L))
        vec.tensor_tensor(out=msq4[:], in0=mu4[:], in1=mu4[:], op=OP.mult)
        vec.tensor_tensor(out=var4[:], in0=ex24[:], in1=msq4[:],
                          op=OP.subtract)
        vec.tensor_scalar_add(out=var4[:], in0=var4[:], scalar1=EPS)
        act.activation(out=std4[:], in_=var4[:], func=AF.Sqrt)
        vec.reciprocal(rstd4[:], std4[:])
        vec.tensor_tensor(out=bscale4[:], in0=rstd4[:], in1=gamma_sb[:],
                          op=OP.mult)
        vec.tensor_tensor(out=tmp4[:], in0=mu4[:], in1=bscale4[:],
                          op=OP.mult)
        vec.tensor_tensor(out=bshift4[:], in0=beta_sb[:], in1=tmp4[:],
                          op=OP.subtract)

    # =============================================================
    # Phase 3+4, one software-pipelined scope: each part's front (raw
    # AllGather -> consumer BN -> in_proj -> dconv -> x_proj -> dbc
    # AllReduce), dt chain, scan, out_proj and ReduceScatter.  Parts
    # p+1's front and dt are emitted INSIDE part p's scan loop so their
    # act/PE work never sits ahead of the scan-feeding ops in any
    # engine's priority order (head-of-line blocking otherwise).
    # CC order: [dummy, statsAR, AG0, AG1, AR0, AG2, AR1, AR2, RS0-2].
    # =============================================================
    scn = contextlib.ExitStack()
    ipp = scn.enter_context(tc.tile_pool(name="ippool", bufs=1))
    ipps = scn.enter_context(
        tc.tile_pool(name="ippsum", bufs=1, space="PSUM"))
    mpps = scn.enter_context(
        tc.tile_pool(name="mppsum", bufs=2, space="PSUM"))
    yps_p = scn.enter_context(
        tc.tile_pool(name="ypsum", bufs=1, space="PSUM"))
    dtp = scn.enter_context(tc.tile_pool(name="dtpool", bufs=1))
    bcp = scn.enter_context(tc.tile_pool(name="bcpool", bufs=6))
    dap = scn.enter_context(tc.tile_pool(name="dapool", bufs=8))
    dbp = scn.enter_context(tc.tile_pool(name="dbpool", bufs=3))
    hsp = scn.enter_context(tc.tile_pool(name="hspool", bufs=3))
    stp = scn.enter_context(tc.tile_pool(name="stpool", bufs=4))
    finp = scn.enter_context(tc.tile_pool(name="finpool", bufs=1))

    winx_sb = ipp.tile([P, 8 * P], BF16, name="winx_sb")
    winz_sb = ipp.tile([P, 8 * P], BF16, name="winz_sb")
    for ci in range(4):
        sync.dma_start(out=winx_sb[:, ci * 2 * P:(ci + 1) * 2 * P],
                       in_=winx.ap()[ci * P:(ci + 1) * P, :])
        sync.dma_start(out=winz_sb[:, ci * 2 * P:(ci + 1) * 2 * P],
                       in_=winz.ap()[ci * P:(ci + 1) * P, :])
    xsp = [ipp.tile([P, L + 3], BF16, name=f"xsp{d}") for d in range(2)]
    for d in range(2):
        gps.memset(xsp[d][:, 0:3], 0.0)

    def emit_front(pp):
        o, sz = PARTS[pp]
        nj = sz // LB
        # schedule-level hold: without it the Tile scheduler hoists the
        # AllGather triggers (inputs ready mid-conv) ahead of the stats
        # AllReduce on the CC, delaying the BN scale/shift chain that
        # gates everything downstream.
        with tc.tile_wait_until(0.07 if pp < 2 else 0.13):
            gps.collective_compute("AllGather", OP.bypass,
                                   replica_groups=GRP_B,
                                   ins=[ng_in_p[pp].opt()],
                                   outs=[ng_out_p[pp].opt()])
        nrm_t = []
        for ci in range(4):
            t = ipp.tile([P, sz], BF16, name=f"nrm{pp}_{ci}",
                         tag=f"nrm_{pp}_{ci}")
            sync.dma_start(out=t[:],
                           in_=ng_out_p[pp][ci * P:(ci + 1) * P, :])
            # in-place batchnorm of the gathered raw conv tile
            act.activation(out=t[:], in_=t[:], func=AF.Identity,
                           scale=bscale4[:, ci:ci + 1],
                           bias=bshift4[:, ci:ci + 1])
            nrm_t.append(t)
        for lb2 in range(nj):
            glb = o // LB + lb2
            for d in range(2):
                ps = ipps.tile([P, LB], F32, name="xzps", tag="xzps")
                for ci in range(4):
                    pe.matmul(out=ps[:],
                              lhsT=winx_sb[:, (ci * 2 + d) * P:
                                           (ci * 2 + d + 1) * P],
                              rhs=nrm_t[ci][:, lb2 * LB:(lb2 + 1) * LB],
                              start=(ci == 0), stop=(ci == 3))
                act.copy(xsp[d][:, 3 + glb * LB:3 + (glb + 1) * LB], ps[:])
                ps2 = ipps.tile([P, LB], F32, name="zps", tag="zps")
                for ci in range(4):
                    pe.matmul(out=ps2[:],
                              lhsT=winz_sb[:, (ci * 2 + d) * P:
                                           (ci * 2 + d + 1) * P],
                              rhs=nrm_t[ci][:, lb2 * LB:(lb2 + 1) * LB],
                              start=(ci == 0), stop=(ci == 3))
                act.activation(out=zsil[d][:, glb * LB:(glb + 1) * LB],
                               in_=ps2[:], func=AF.Silu)
        # depthwise causal conv: 4 diagonal matmuls per (d, lb2)
        for d in range(2):
            for lb2 in range(nj):
                glb = o // LB + lb2
                ps3 = mpps.tile([P, LB], F32, name="mps", tag="mps")
                for k in range(DC):
                    jj = d * DC + k
                    pe.matmul(
                        out=ps3[:],
                        lhsT=convd_sb[:, jj * P:(jj + 1) * P],
                        rhs=xsp[d][:, k + glb * LB:k + glb * LB + LB],
                        start=(k == 0), stop=(k == DC - 1))
                act.activation(out=xs_act[d][:, glb * LB:(glb + 1) * LB],
                               in_=ps3[:], func=AF.Silu,
                               bias=convb_sb[:, d:d + 1])
        # x_proj partial -> AllReduce within b-group
        dbc_sb = ipp.tile([RK + 2 * N, sz], BF16, name=f"dbc_sb{pp}",
                          tag=f"dbc_sb{pp}")
        for j in range(nj):
            ps = mpps.tile([P, LB], F32, name="mps", tag="mps")
            for d in range(2):
                pe.matmul(out=ps[0:RK + 2 * N, :],
                          lhsT=wxp_sb[:, d * 64:(d + 1) * 64],
                          rhs=xs_act[d][:, o + j * LB:o + (j + 1) * LB],
                          start=(d == 0), stop=(d == 1))
            act.copy(dbc_sb[:, j * LB:(j + 1) * LB], ps[0:RK + 2 * N, :])
        act.dma_start(out=dbc_in_p[pp][:], in_=dbc_sb[:])
        gps.collective_compute("AllReduce", OP.add,
                               replica_groups=GRP_B,
                               ins=[dbc_in_p[pp].opt()],
                               outs=[dbc_out_p[pp].opt()])

    def emit_dt(part):
        """x_proj AllReduce result -> dt (softplus): PE + act stages."""
        o, sz = PARTS[part]
        nj = sz // LB
        dtr = dtp.tile([RK, sz], BF16, name=f"dtr{part}", tag=f"dtr{part}")
        # part 0: the sync queue is idle exactly when AR0 lands, so its
        # trigger fires immediately (act is still draining front silus).
        # parts 1-2: act, whose next ops are this chain's own Exps —
        # sync's stream buries the trigger behind bc-broadcast triggers.
        (sync if part == 0 else act).dma_start(
            out=dtr[:], in_=dbc_out_p[part][0:RK, :])
        ets = []
        for d in range(2):
            for j in range(nj):
                ps = mpps.tile([P, LB], F32, name="mps", tag="mps")
                pe.matmul(out=ps[:],
                          lhsT=wdt_sb[:, d * P:(d + 1) * P],
                          rhs=dtr[:, j * LB:(j + 1) * LB],
                          start=True, stop=True)
                # softplus(x) = ln(1 + exp(x)); Exp then Ln batched so the
                # act table set switches only twice.
                et = dtp.tile([P, LB], F32, name="et", bufs=4)
                act.activation(out=et[:], in_=ps[:], func=AF.Exp,
                               bias=dtb_sb[:, d:d + 1])
                ets.append((d, j, et))
        for d, j, et in ets:
            act.activation(
                out=dtT[d][:, o + j * LB:o + (j + 1) * LB],
                in_=et[:], func=AF.Ln, bias=1.0)

    def emit_vt(part):
        """v = dt * xs on DVE — separate so it can sit later in the vec
        queue than the act-side dt chain sits in the act queue."""
        o, sz = PARTS[part]
        for d in range(2):
            vec.tensor_tensor(out=vT[d][:, o:o + sz],
                              in0=dtT[d][:, o:o + sz],
                              in1=xs_act[d][:, o:o + sz], op=OP.mult)

    def emit_outproj(part):
        """out_proj partials for one part, staged to DRAM for the RS."""
        o, sz = PARTS[part]
        nj = sz // LB
        att_h = att_in_p[part]
        for j in range(nj):
            glb = o // LB + j
            for ct in range(4):
                ps = mpps.tile([P, LB], F32, name="mps", tag="mps")
                for d in range(2):
                    pe.matmul(
                        out=ps[:],
                        lhsT=wout_sb[:, (d * 4 + ct) * P:
                                     (d * 4 + ct + 1) * P],
                        rhs=yg[d][:, glb * LB:(glb + 1) * LB],
                        start=(d == 0), stop=(d == 1))
                st = stp.tile([P, LB], BF16, name="atstage")
                act.copy(st[:], ps[:])
                act.dma_start(
                    out=att_h[ct * P:(ct + 1) * P, j * LB:(j + 1) * LB],
                    in_=st[:])

    def emit_rs(part):
        gps.collective_compute("ReduceScatter", OP.add,
                               replica_groups=GRP_B,
                               ins=[att_in_p[part].opt()],
                               outs=[att_out_p[part].opt()])

    def emit_scan(part, hooks):
        o, sz = PARTS[part]
        nj = sz // LB
        yps = [[yps_p.tile([P, LB], F32, name=f"yps{d}_{j}",
                           tag=f"yps{d}_{j}") for j in range(nj)]
               for d in range(2)]
        # open the PSUM accumulation with the D_skip * xs diagonal term
        for d in range(2):
            for j in range(nj):
                pe.matmul(out=yps[d][j][:],
                          lhsT=dskipd_sb[:, d * P:(d + 1) * P],
                          rhs=xs_act[d][:, o + j * LB:o + (j + 1) * LB],
                          start=True, stop=False, skip_group_check=True)
        for n in range(N):
            if n in hooks:
                hooks[n]()
            if n == N - 1 and "post" in hooks:
                # next part's dt chain goes right before the last scan
                # unit: late enough that the act queue reaches it after
                # this part's exps, early enough to be done when the
                # next part's scans start.
                hooks["post"]()
            # B/C rows broadcast to 128 partitions via stride-0 DMA reads.
            # Fixed-size allocs (sliced per part) keep pool tags uniform;
            # 4 KiB-aligned ring buffers so the act engine's write of
            # da[k+few] never lands in the bank DVE is reading da[k] from.
            bc = bcp.tile([P, 2048], BF16, name="bc")
            sync.dma_start(
                out=bc[:, 0:sz],
                in_=dbc_out_p[part][RK + n:RK + n + 1, :]
                .to_broadcast((P, sz)))
            act.dma_start(
                out=bc[:, 1024:1024 + sz],
                in_=dbc_out_p[part][RK + N + n:RK + N + n + 1, :]
                .to_broadcast((P, sz)))
            for d in range(2):
                idx = n * 2 + d
                da = dap.tile([P, 1024], BF16, name="da",
                              padded_shape=[P, 2048])
                act.activation(
                    out=da[:, 0:sz], in_=dtT[d][:, o:o + sz], func=AF.Exp,
                    scale=acol_sb[:, d * N + n:d * N + n + 1])
                dbx = dbp.tile([P, 1024], BF16, name="dbx",
                               padded_shape=[P, 2048])
                vec.tensor_tensor(out=dbx[:, 0:sz], in0=vT[d][:, o:o + sz],
                                  in1=bc[:, 0:sz], op=OP.mult)
                hs = hsp.tile([P, 1024], BF16, name="hs",
                              padded_shape=[P, 2048])
                vec.tensor_tensor_scan(
                    out=hs[:, 0:sz], data0=da[:, 0:sz], data1=dbx[:, 0:sz],
                    initial=(0.0 if part == 0 else hfin[:, idx:idx + 1]),
                    op0=OP.mult, op1=OP.add)
                if part < NPART - 1:
                    vec.tensor_copy(hfin[:, idx:idx + 1], hs[:, sz - 1:sz])
                vec.tensor_tensor(out=dbx[:, 0:sz], in0=hs[:, 0:sz],
                                  in1=bc[:, 1024:1024 + sz], op=OP.mult)
                for j in range(nj):
                    pe.matmul(out=yps[d][j][:], lhsT=idt_sb[:],
                              rhs=dbx[:, j * LB:(j + 1) * LB],
                              start=False, stop=(n == N - 1),
                              skip_group_check=True)
        # gate with silu(z) straight out of PSUM
        for d in range(2):
            for j in range(nj):
                glb = o // LB + j
                vec.tensor_tensor(out=yg[d][:, glb * LB:(glb + 1) * LB],
                                  in0=yps[d][j][:],
                                  in1=zsil[d][:, glb * LB:(glb + 1) * LB],
                                  op=OP.mult)

    # ---- the part pipeline: part p+1's front and dt, and part p-1's
    # out_proj/RS, are emitted inside part p's scan loop so every
    # engine queue's priority order matches the execution order.
    emit_front(0)
    emit_dt(0)
    emit_vt(0)
    for pp in range(NPART):
        hooks = {}
        if pp + 1 < NPART:
            hooks[2] = (lambda q: lambda: emit_front(q))(pp + 1)
            hooks[12] = (lambda q: lambda: emit_dt(q))(pp + 1)
            hooks["post"] = (lambda q: lambda: emit_vt(q))(pp + 1)
        if pp > 0:
            hooks[6] = (lambda q: lambda: emit_outproj(q))(pp - 1)
            hooks[10] = (lambda q: lambda: emit_rs(q))(pp - 1)
        emit_scan(pp, hooks)

    # ---- last part's out_proj + RS, then all residual tails
    emit_outproj(NPART - 1)
    emit_rs(NPART - 1)
    for pp, (o, sz) in enumerate(PARTS):
        att_sb = finp.tile([CT, sz], BF16, name=f"att_sb{pp}")
        out_sb = finp.tile([CT, sz], F32, name=f"out_sb{pp}")
        sync.dma_start(out=att_sb[:], in_=att_out_p[pp][:])
        vec.tensor_tensor(out=out_sb[:], in0=att_sb[:],
                          in1=xct_sb[:, o:o + sz], op=OP.add)
        sync.dma_start(out=out.ap()[:, o:o + sz], in_=out_sb[:])

    scn.close()
    _ctx.close()


_NC_CACHE = None
LAST_EXEC_NS = None


def _get_nc():
    global _NC_CACHE
    if _NC_CACHE is None:
        _NC_CACHE = build_graph()
    return _NC_CACHE


def make_in_maps(inputs):
    f32 = lambda a: np.ascontiguousarray(np.asarray(a), dtype=np.float32)
    bf16 = lambda a: np.ascontiguousarray(
        np.asarray(a, dtype=np.float32).astype(ml_dtypes.bfloat16))
    x = f32(inputs["x"])
    ff_w = f32(inputs["ff_w"])
    ff_b = f32(inputs["ff_b"])
    g = f32(inputs["bn_gamma"])
    bt = f32(inputs["bn_beta"])
    w_in = f32(inputs["w_in"])
    conv_w = f32(inputs["conv_w"])
    conv_b = f32(inputs["conv_b"])
    w_xproj = f32(inputs["w_xproj"])
    w_dt = f32(inputs["w_dt"])
    dt_bias = f32(inputs["dt_bias"])
    A = -np.exp(f32(inputs["A_log"]))
    D_skip = f32(inputs["D_skip"])
    w_out = f32(inputs["w_out"])
    ffw_t = np.transpose(ff_w, (2, 1, 0))  # [K, C, co]

    in_maps = []
    for i in range(8):
        b, q = i // 4, i % 4
        dsl = slice(q * S, (q + 1) * S)
        csl = slice(q * CT, (q + 1) * CT)
        dsk = D_skip[dsl]
        dskd = np.stack([np.diag(dsk[dd * P:(dd + 1) * P])
                         for dd in range(2)]).reshape(S, P)
        in_maps.append({
            "xb": bf16(x[b]),
            "xct": f32(x[b, csl]),
            "ffw": bf16(ffw_t[:, :, csl]),
            "ffb": f32(ff_b[csl].reshape(CT, 1)),
            "gamma": f32(g.reshape(4, CT).T),
            "beta": f32(bt.reshape(4, CT).T),
            "qsel": f32((np.arange(4)[None, :] == q).astype(np.float32)
                        * np.ones((CT, 1), np.float32)),
            "winx": bf16(w_in[:, :DI][:, dsl]),
            "winz": bf16(w_in[:, DI:][:, dsl]),
            "convd": bf16(np.stack([
                np.stack([np.diag(conv_w[dsl][dd * P:(dd + 1) * P, k])
                          for dd in range(2)]).reshape(S, P)
                for k in range(DC)])),
            "convb": f32(conv_b[dsl].reshape(S, 1)),
            "wxp": bf16(w_xproj[dsl]),
            "wdt": bf16(w_dt[:, dsl]),
            "dtb": f32(dt_bias[dsl].reshape(S, 1)),
            "acol": f32(A[dsl]),
            "dskipd": bf16(dskd),
            "wout": bf16(w_out[dsl]),
            "ident": np.eye(P, dtype=np.float32).astype(ml_dtypes.bfloat16),
        })
    return in_maps


def _install_ntff_hook():
    """The agent image's antenv lacks axon_hooks; recreate it so
    run_bass_kernel_spmd(trace=True) can NTFF-profile via the axon .so."""
    import types
    if "antenv.axon_hooks" in sys.modules:
        return
    try:
        from trn_agent_boot.trn_boot import _ntff_profile_via_ctypes
        hook = _ntff_profile_via_ctypes("/opt/axon/libaxon_pjrt.so")
    except Exception:
        hook = None
    mod = types.ModuleType("antenv.axon_hooks")
    mod.get_axon_ntff_profile_hook = lambda: hook
    mod.set_axon_ntff_profile_hook = lambda h: None
    sys.modules["antenv.axon_hooks"] = mod


def kernel(**inputs):
    global LAST_EXEC_NS
    nc = _get_nc()
    in_maps = make_in_maps(inputs)
    trace = os.environ.get("KERNEL_TRACE", "0") == "1"
    if trace:
        _install_ntff_hook()
    try:
        res = run_bass_kernel_spmd(nc, in_maps, core_ids=list(range(8)),
                                   trace=trace)
    except Exception:
        if not trace:
            raise
        res = run_bass_kernel_spmd(nc, in_maps, core_ids=list(range(8)),
                                   trace=False)
    LAST_EXEC_NS = res.exec_time_ns
    out = np.empty((B, C, L), dtype=np.float32)
    for i in range(8):
        b, q = i // 4, i % 4
        out[b, q * CT:(q + 1) * CT] = res.results[i]["out"]
    return out
